# revision 4
# baseline (speedup 1.0000x reference)
"""Trainium2 kernel for nn_DifferentiableSuperpixelTokenizer (segment mean of
linearly-projected pixel features).

Identity used: segment_mean(concat(img, xy) @ W + b) can be computed as
segment sums of SIX small per-pixel features (r, g, b, x, y, 1) followed by a
tiny [196, 6] @ [6, 768] projection, because the projection is linear:

    out[s] = (segsum_feat6[s] / clamp(counts[s], 1)) @ [W; b]

Sharding: data-parallel over batch (8 batches -> 8 NeuronCores), exactly as the
per-batch segment-id offsets intend; each core reduces its own batch into its
own [196, 768] slice, host stacks them.

Per core the segment reduction runs as 392 chunks of 128 pixels:
  - DVE/ACT/GPSIMD build a [128, 196] fp16 one-hot per chunk (tensor_scalar
    is_equal against a constant iota row, or Square+Relu(1-d^2) on ACT)
  - TensorE accumulates feat_chunk[128, 6].T @ onehot[128, 196] into PSUM,
    4 chunks concurrently via col-tiling (tile_position=(0, 32j))
"""
import numpy as np
from contextlib import ExitStack

# ---------------------------------------------------------------------------
# Workarounds for walrus codegen supporting a single sem-wait per instruction
# ---------------------------------------------------------------------------
import bass_rust
import concourse.mybir as mybir
import concourse.tile as tile
from concourse.tile import ScopedClock

_MAX_INST_WAITS = 1
_SELF_DROP_ENGINES = {
    mybir.EngineType.DVE: "DVE",
    mybir.EngineType.Activation: "Activation",
    mybir.EngineType.Pool: "Pool",
}


def _split_waits(ins):
    si = getattr(ins, "sync_info", None)
    if si is None:
        return []
    waits = list(si.on_wait)
    if not waits:
        return []
    self_name = _SELF_DROP_ENGINES.get(ins.engine)
    if self_name is not None:
        kept = [w for w in waits if w.ant_name.rsplit("_", 1)[0] != self_name]
    else:
        kept = waits
    head = kept[:-_MAX_INST_WAITS] if len(kept) > _MAX_INST_WAITS else []
    rest = kept[len(head):]
    if len(waits) != len(rest) or head:
        ins.sync_info = bass_rust.SyncInfo(on_wait=rest, on_update=list(si.on_update))
    return head


_orig_commit = tile.TileContext._commit_instruction


def _patched_commit(self, inst, lazy_reg_writes=True):
    head = _split_waits(inst)
    for i in range(0, len(head), _MAX_INST_WAITS):
        nop = mybir.InstNoOp(
            name=self.nc.get_next_instruction_name(),
            sync_info=mybir.SyncInfo(
                on_wait=head[i : i + _MAX_INST_WAITS], on_update=[]
            ),
            bass_nofuse=True,
            engine=inst.engine,
        )
        _orig_commit(self, nop, lazy_reg_writes=False)
    return _orig_commit(self, inst, lazy_reg_writes)


def _patched_drain_and_barrier(self, tick_clock, wait_clock):
    nc = self.nc
    drain_inst = nc.sync.drain()
    wait_clock.add_sem_waits(
        drain_inst.ins, ScopedClock({None: tick_clock.global_clock})
    )
    si = drain_inst.ins.sync_info
    waits = list(si.on_wait) if si is not None else []
    if len(waits) > 1:
        drain_inst.ins.sync_info = bass_rust.SyncInfo(on_wait=waits[:1], on_update=[])
        for w in waits[1:]:
            d2 = nc.sync.drain()
            d2.ins.sync_info = bass_rust.SyncInfo(on_wait=[w], on_update=[])
    nc.all_engine_barrier()
    assert self.sems is not None
    popped = nc._tile_sem_poison_stack.pop()
    assert popped is self._sem_poison
    nc.clear_and_free_semaphores(list(self.sems.allocated().values()))


tile.TileContext._drain_and_barrier = _patched_drain_and_barrier
tile.TileContext._commit_instruction = _patched_commit

import concourse.bass as bass  # noqa: E402

# ---------------------------------------------------------------------------
# Kernel
# ---------------------------------------------------------------------------
P, T, S, E, F = 128, 392, 196, 768, 6
H = Wimg = 224
B = 8

FP16 = mybir.dt.float16
F32 = mybir.dt.float32

PATTERN = ("dve",)
COL_TILE = True
OH_BUFS = 8


def _make_coords():
    x = np.arange(Wimg, dtype=np.float32) / np.float32(Wimg - 1)
    y = np.arange(H, dtype=np.float32) / np.float32(H - 1)
    xg = np.broadcast_to(x[None, :], (H, Wimg))
    yg = np.broadcast_to(y[:, None], (H, Wimg))
    return np.stack([xg.ravel(), yg.ravel()])  # [2, N] (x, y)


def _prep_core_inputs(img, segments, W, b):
    coords = _make_coords().reshape(2, P, T)
    ones = np.ones((1, P, T), np.float32)
    w6 = np.ascontiguousarray(np.concatenate([W, b[None, :]], 0).astype(np.float32))
    maps = []
    for bi in range(B):
        imgb = img[bi].reshape(3, P, T).astype(np.float32)
        feat6 = np.concatenate([imgb, coords, ones], 0)  # [6, P, T]
        feat_host = np.ascontiguousarray(
            feat6.transpose(1, 2, 0).reshape(P, T * F)
        ).astype(np.float16)
        seg_host = np.ascontiguousarray(segments[bi].reshape(P, T)).astype(np.float32)
        maps.append({"feat": feat_host, "seg": seg_host, "w6": w6})
    return maps


def _build_program(col_tile=COL_TILE, oh_bufs=OH_BUFS, pattern=PATTERN):
    nc = bass.Bass("TRN2", debug=False)
    feat = nc.dram_tensor("feat", [P, T * F], FP16, kind="ExternalInput")
    seg = nc.dram_tensor("seg", [P, T], F32, kind="ExternalInput")
    w6 = nc.dram_tensor("w6", [F, E], F32, kind="ExternalInput")
    out = nc.dram_tensor("out", [S, E], F32, kind="ExternalOutput")

    ngroups = 4 if col_tile else 1
    iota_np = np.ascontiguousarray(
        np.broadcast_to(np.arange(S, dtype=np.float16), (P, S))
    )
    iota_c = nc.inline_tensor(iota_np, name="iota_const")
    comb_np = np.zeros((P, F), np.float32)
    for j in range(ngroups):
        comb_np[32 * j : 32 * j + F, :] = np.eye(F, dtype=np.float32)
    comb_c = nc.inline_tensor(comb_np, name="comb_const")
    sel6_np = np.zeros((F, F), np.float32)
    sel6_np[F - 1, :] = 1.0
    sel6_c = nc.inline_tensor(sel6_np, name="sel6_const")

    with tile.TileContext(nc) as tc, ExitStack() as ctx:
        sb = ctx.enter_context(tc.tile_pool(name="sb", bufs=1))
        ohp = ctx.enter_context(tc.tile_pool(name="oh", bufs=oh_bufs))
        pp = ctx.enter_context(tc.tile_pool(name="psum", bufs=1, space="PSUM"))

        seg_sb = sb.tile([P, T], F32)
        nc.sync.dma_start(out=seg_sb[:], in_=seg.ap()[:, :])
        iota_sb = sb.tile([P, S], FP16)
        nc.sync.dma_start(out=iota_sb[:], in_=iota_c.ap()[:, :])
        feat_sb = sb.tile([P, T * F], FP16)
        for q in range(4):
            qs = q * (T * F // 4)
            qe = (q + 1) * (T * F // 4)
            nc.sync.dma_start(out=feat_sb[:, qs:qe], in_=feat.ap()[:, qs:qe])
        w6_sb = sb.tile([F, E], F32)
        nc.sync.dma_start(out=w6_sb[:], in_=w6.ap()[:, :])
        comb_sb = sb.tile([P, F], F32)
        nc.sync.dma_start(out=comb_sb[:], in_=comb_c.ap()[:, :])
        sel6_sb = sb.tile([F, F], F32)
        nc.sync.dma_start(out=sel6_sb[:], in_=sel6_c.ap()[:, :])

        use_act = "act" in pattern
        if use_act:
            nseg_sb = sb.tile([P, T], F32)
            nc.vector.tensor_scalar(
                out=nseg_sb[:],
                in0=seg_sb[:],
                scalar1=-1.0,
                scalar2=None,
                op0=mybir.AluOpType.mult,
            )

        acc = pp.tile([P, S], F32)
        # zero rows the col-tiled matmuls never touch (comb reads all 128
        # rows; PSUM garbage there can be NaN and 0*NaN = NaN)
        nc.vector.memset(acc[:], 0.0)
        for t in range(T):
            j = t % ngroups
            eng = pattern[t % len(pattern)]
            oh = ohp.tile([P, S], FP16, tag=f"oh_{eng}")
            if eng == "dve":
                nc.vector.tensor_scalar(
                    out=oh[:],
                    in0=iota_sb[:],
                    scalar1=seg_sb[:, t : t + 1],
                    scalar2=None,
                    op0=mybir.AluOpType.is_equal,
                )
            elif eng == "gpsimd":
                nc.gpsimd.tensor_scalar(
                    out=oh[:],
                    in0=iota_sb[:],
                    scalar1=seg_sb[:, t : t + 1],
                    scalar2=None,
                    op0=mybir.AluOpType.is_equal,
                )
            elif eng == "act":
                d2 = ohp.tile([P, S], FP16, tag="d2")
                nc.scalar.activation(
                    d2[:],
                    iota_sb[:],
                    mybir.ActivationFunctionType.Square,
                    bias=nseg_sb[:, t : t + 1],
                )
                nc.scalar.activation(
                    oh[:],
                    d2[:],
                    mybir.ActivationFunctionType.Relu,
                    bias=1.0,
                    scale=-1.0,
                )
            else:
                raise ValueError(eng)
            nc.tensor.matmul(
                acc[32 * j : 32 * j + F, :],
                lhsT=feat_sb[:, F * t : F * (t + 1)],
                rhs=oh[:],
                start=(t < ngroups),
                stop=(t >= T - ngroups),
                tile_position=(0, 32 * j) if col_tile else None,
            )

        acc_sb = sb.tile([P, S], F32)
        nc.vector.tensor_copy(acc_sb[:], acc[:])
        sums_psum = pp.tile([F, S], F32)
        nc.tensor.matmul(
            sums_psum[:], lhsT=comb_sb[:], rhs=acc_sb[:], start=True, stop=True
        )

        cl_sb = sb.tile([F, S], F32)
        nc.vector.tensor_scalar(
            out=cl_sb[:],
            in0=sums_psum[:, :],
            scalar1=1.0,
            scalar2=None,
            op0=mybir.AluOpType.max,
        )
        recip_sb = sb.tile([F, S], F32)
        nc.vector.reciprocal(recip_sb[:], cl_sb[:])

        rb_psum = pp.tile([F, S], F32)
        nc.tensor.matmul(
            rb_psum[:], lhsT=sel6_sb[:], rhs=recip_sb[:], start=True, stop=True
        )
        sums_sb = sb.tile([F, S], F32)
        nc.vector.tensor_copy(sums_sb[:], sums_psum[:])
        mean_sb = sb.tile([F, S], F32)
        nc.vector.tensor_tensor(
            mean_sb[:], sums_sb[:], rb_psum[:], op=mybir.AluOpType.mult
        )

        for lo, hi in ((0, P), (P, S)):
            m = hi - lo
            optile = pp.tile([P, E], F32, tag=f"op{lo}")
            nc.tensor.matmul(
                optile[:m, 0:512],
                lhsT=mean_sb[:, lo:hi],
                rhs=w6_sb[:, 0:512],
                start=True,
                stop=True,
            )
            nc.tensor.matmul(
                optile[:m, 512:E],
                lhsT=mean_sb[:, lo:hi],
                rhs=w6_sb[:, 512:E],
                start=True,
                stop=True,
            )
            ob = sb.tile([P, E], F32, tag=f"ob{lo}")
            nc.any.tensor_copy(ob[:m, :], optile[:m, :])
            nc.sync.dma_start(out=out.ap()[lo:hi, :], in_=ob[:m, :])

    return nc


_PROGRAM_CACHE = {}


def kernel(**inputs) -> np.ndarray:
    from concourse.bass_utils import run_bass_kernel_spmd

    img = np.asarray(inputs["img"]).astype(np.float32)
    segments = np.asarray(inputs["segments"])
    W = np.asarray(inputs["W"]).astype(np.float32)
    b = np.asarray(inputs["b"]).astype(np.float32)

    in_maps = _prep_core_inputs(img, segments, W, b)
    key = (COL_TILE, OH_BUFS, PATTERN)
    if key not in _PROGRAM_CACHE:
        _PROGRAM_CACHE[key] = _build_program()
    nc = _PROGRAM_CACHE[key]
    res = run_bass_kernel_spmd(nc, in_maps, list(range(B)))
    out = np.stack([res.results[i]["out"] for i in range(B)]).astype(np.float32)
    return out


# revision 5
# speedup vs baseline: 4.3598x; 4.3598x over previous
"""Trainium2 kernel for nn_DifferentiableSuperpixelTokenizer (segment mean of
linearly-projected pixel features).

Identity used: segment_mean(concat(img, xy) @ W + b) can be computed as
segment sums of SIX small per-pixel features (r, g, b, x, y, 1) followed by a
tiny [196, 6] @ [6, 768] projection, because the projection is linear:

    out[s] = (segsum_feat6[s] / clamp(counts[s], 1)) @ [W; b]

Sharding: data-parallel over batch (8 batches -> 8 NeuronCores), exactly as the
per-batch segment-id offsets intend; each core reduces its own batch into its
own [196, 768] slice, host stacks them.

Per core the segment reduction runs as 392 chunks of 128 pixels:
  - DVE/ACT/GPSIMD build a [128, 196] fp16 one-hot per chunk (tensor_scalar
    is_equal against a constant iota row, or Square+Relu(1-d^2) on ACT)
  - TensorE accumulates feat_chunk[128, 6].T @ onehot[128, 196] into PSUM,
    4 chunks concurrently via col-tiling (tile_position=(0, 32j))
"""
import numpy as np
from contextlib import ExitStack

# ---------------------------------------------------------------------------
# Workarounds for walrus codegen supporting a single sem-wait per instruction
# ---------------------------------------------------------------------------
import bass_rust
import concourse.mybir as mybir
import concourse.tile as tile
from concourse.tile import ScopedClock

_MAX_INST_WAITS = 1
_SELF_DROP_ENGINES = {
    mybir.EngineType.DVE: "DVE",
    mybir.EngineType.Activation: "Activation",
    mybir.EngineType.Pool: "Pool",
}


def _split_waits(ins):
    si = getattr(ins, "sync_info", None)
    if si is None:
        return []
    waits = list(si.on_wait)
    if not waits:
        return []
    self_name = _SELF_DROP_ENGINES.get(ins.engine)
    if self_name is not None:
        kept = [w for w in waits if w.ant_name.rsplit("_", 1)[0] != self_name]
    else:
        kept = waits
    head = kept[:-_MAX_INST_WAITS] if len(kept) > _MAX_INST_WAITS else []
    rest = kept[len(head):]
    if len(waits) != len(rest) or head:
        ins.sync_info = bass_rust.SyncInfo(on_wait=rest, on_update=list(si.on_update))
    return head


_orig_commit = tile.TileContext._commit_instruction


def _patched_commit(self, inst, lazy_reg_writes=True):
    head = _split_waits(inst)
    for i in range(0, len(head), _MAX_INST_WAITS):
        nop = mybir.InstNoOp(
            name=self.nc.get_next_instruction_name(),
            sync_info=mybir.SyncInfo(
                on_wait=head[i : i + _MAX_INST_WAITS], on_update=[]
            ),
            bass_nofuse=True,
            engine=inst.engine,
        )
        _orig_commit(self, nop, lazy_reg_writes=False)
    return _orig_commit(self, inst, lazy_reg_writes)


def _patched_drain_and_barrier(self, tick_clock, wait_clock):
    nc = self.nc
    drain_inst = nc.sync.drain()
    wait_clock.add_sem_waits(
        drain_inst.ins, ScopedClock({None: tick_clock.global_clock})
    )
    si = drain_inst.ins.sync_info
    waits = list(si.on_wait) if si is not None else []
    if len(waits) > 1:
        drain_inst.ins.sync_info = bass_rust.SyncInfo(on_wait=waits[:1], on_update=[])
        for w in waits[1:]:
            d2 = nc.sync.drain()
            d2.ins.sync_info = bass_rust.SyncInfo(on_wait=[w], on_update=[])
    nc.all_engine_barrier()
    assert self.sems is not None
    popped = nc._tile_sem_poison_stack.pop()
    assert popped is self._sem_poison
    nc.clear_and_free_semaphores(list(self.sems.allocated().values()))


tile.TileContext._drain_and_barrier = _patched_drain_and_barrier
tile.TileContext._commit_instruction = _patched_commit

import concourse.bass as bass  # noqa: E402

# ---------------------------------------------------------------------------
# Kernel
# ---------------------------------------------------------------------------
P, T, S, E, F = 128, 392, 196, 768, 6
H = Wimg = 224
B = 8

FP16 = mybir.dt.float16
F32 = mybir.dt.float32

PATTERN = ("dve", "dve", "dve", "dve", "dve", "act")
COL_TILE = True
OH_BUFS = 8


def _make_coords():
    x = np.arange(Wimg, dtype=np.float32) / np.float32(Wimg - 1)
    y = np.arange(H, dtype=np.float32) / np.float32(H - 1)
    xg = np.broadcast_to(x[None, :], (H, Wimg))
    yg = np.broadcast_to(y[:, None], (H, Wimg))
    return np.stack([xg.ravel(), yg.ravel()])  # [2, N] (x, y)


def _prep_core_inputs(img, segments, W, b):
    coords = _make_coords().reshape(2, P, T)
    ones = np.ones((1, P, T), np.float32)
    w6 = np.ascontiguousarray(np.concatenate([W, b[None, :]], 0).astype(np.float32))
    maps = []
    for bi in range(B):
        imgb = img[bi].reshape(3, P, T).astype(np.float32)
        feat6 = np.concatenate([imgb, coords, ones], 0)  # [6, P, T]
        feat_host = np.ascontiguousarray(
            feat6.transpose(1, 2, 0).reshape(P, T * F)
        ).astype(np.float16)
        seg_host = np.ascontiguousarray(segments[bi].reshape(P, T)).astype(np.float32)
        maps.append({"feat": feat_host, "seg": seg_host, "w6": w6})
    return maps


def _build_program(col_tile=COL_TILE, oh_bufs=OH_BUFS, pattern=PATTERN):
    nc = bass.Bass("TRN2", debug=False)
    feat = nc.dram_tensor("feat", [P, T * F], FP16, kind="ExternalInput")
    seg = nc.dram_tensor("seg", [P, T], F32, kind="ExternalInput")
    w6 = nc.dram_tensor("w6", [F, E], F32, kind="ExternalInput")
    out = nc.dram_tensor("out", [S, E], F32, kind="ExternalOutput")

    ngroups = 4 if col_tile else 1
    iota_np = np.ascontiguousarray(
        np.broadcast_to(np.arange(S, dtype=np.float16), (P, S))
    )
    iota_c = nc.inline_tensor(iota_np, name="iota_const")
    comb_np = np.zeros((P, F), np.float32)
    for j in range(ngroups):
        comb_np[32 * j : 32 * j + F, :] = np.eye(F, dtype=np.float32)
    comb_c = nc.inline_tensor(comb_np, name="comb_const")
    sel6_np = np.zeros((F, F), np.float32)
    sel6_np[F - 1, :] = 1.0
    sel6_c = nc.inline_tensor(sel6_np, name="sel6_const")

    with tile.TileContext(nc) as tc, ExitStack() as ctx:
        sb = ctx.enter_context(tc.tile_pool(name="sb", bufs=1))
        ohp = ctx.enter_context(tc.tile_pool(name="oh", bufs=oh_bufs))
        pp = ctx.enter_context(tc.tile_pool(name="psum", bufs=1, space="PSUM"))

        seg_sb = sb.tile([P, T], F32)
        nc.sync.dma_start(out=seg_sb[:], in_=seg.ap()[:, :])
        iota_sb = sb.tile([P, S], FP16)
        nc.sync.dma_start(out=iota_sb[:], in_=iota_c.ap()[:, :])
        feat_sb = sb.tile([P, T * F], FP16)
        for q in range(4):
            qs = q * (T * F // 4)
            qe = (q + 1) * (T * F // 4)
            nc.sync.dma_start(out=feat_sb[:, qs:qe], in_=feat.ap()[:, qs:qe])
        w6_sb = sb.tile([F, E], F32)
        nc.sync.dma_start(out=w6_sb[:], in_=w6.ap()[:, :])
        comb_sb = sb.tile([P, F], F32)
        nc.sync.dma_start(out=comb_sb[:], in_=comb_c.ap()[:, :])
        sel6_sb = sb.tile([F, F], F32)
        nc.sync.dma_start(out=sel6_sb[:], in_=sel6_c.ap()[:, :])

        use_act = "act" in pattern
        if use_act:
            nseg_sb = sb.tile([P, T], F32)
            nc.vector.tensor_scalar(
                out=nseg_sb[:],
                in0=seg_sb[:],
                scalar1=-1.0,
                scalar2=None,
                op0=mybir.AluOpType.mult,
            )

        acc = pp.tile([P, S], F32)
        # zero rows the col-tiled matmuls never touch (comb reads all 128
        # rows; PSUM garbage there can be NaN and 0*NaN = NaN)
        nc.vector.memset(acc[:], 0.0)
        for t in range(T):
            j = t % ngroups
            eng = pattern[t % len(pattern)]
            oh = ohp.tile([P, S], FP16, tag=f"oh_{eng}")
            if eng == "dve":
                nc.vector.tensor_scalar(
                    out=oh[:],
                    in0=iota_sb[:],
                    scalar1=seg_sb[:, t : t + 1],
                    scalar2=None,
                    op0=mybir.AluOpType.is_equal,
                )
            elif eng == "gpsimd":
                nc.gpsimd.tensor_scalar(
                    out=oh[:],
                    in0=iota_sb[:],
                    scalar1=seg_sb[:, t : t + 1],
                    scalar2=None,
                    op0=mybir.AluOpType.is_equal,
                )
            elif eng == "act":
                d2 = ohp.tile([P, S], FP16, tag="d2")
                nc.scalar.activation(
                    d2[:],
                    iota_sb[:],
                    mybir.ActivationFunctionType.Square,
                    bias=nseg_sb[:, t : t + 1],
                )
                nc.scalar.activation(
                    oh[:],
                    d2[:],
                    mybir.ActivationFunctionType.Relu,
                    bias=1.0,
                    scale=-1.0,
                )
            else:
                raise ValueError(eng)
            nc.tensor.matmul(
                acc[32 * j : 32 * j + F, :],
                lhsT=feat_sb[:, F * t : F * (t + 1)],
                rhs=oh[:],
                start=(t < ngroups),
                stop=(t >= T - ngroups),
                tile_position=(0, 32 * j) if col_tile else None,
            )

        acc_sb = sb.tile([P, S], F32)
        nc.vector.tensor_copy(acc_sb[:], acc[:])
        sums_psum = pp.tile([F, S], F32)
        nc.tensor.matmul(
            sums_psum[:], lhsT=comb_sb[:], rhs=acc_sb[:], start=True, stop=True
        )

        cl_sb = sb.tile([F, S], F32)
        nc.vector.tensor_scalar(
            out=cl_sb[:],
            in0=sums_psum[:, :],
            scalar1=1.0,
            scalar2=None,
            op0=mybir.AluOpType.max,
        )
        recip_sb = sb.tile([F, S], F32)
        nc.vector.reciprocal(recip_sb[:], cl_sb[:])

        rb_psum = pp.tile([F, S], F32)
        nc.tensor.matmul(
            rb_psum[:], lhsT=sel6_sb[:], rhs=recip_sb[:], start=True, stop=True
        )
        sums_sb = sb.tile([F, S], F32)
        nc.vector.tensor_copy(sums_sb[:], sums_psum[:])
        mean_sb = sb.tile([F, S], F32)
        nc.vector.tensor_tensor(
            mean_sb[:], sums_sb[:], rb_psum[:], op=mybir.AluOpType.mult
        )

        for lo, hi in ((0, P), (P, S)):
            m = hi - lo
            optile = pp.tile([P, E], F32, tag=f"op{lo}")
            nc.tensor.matmul(
                optile[:m, 0:512],
                lhsT=mean_sb[:, lo:hi],
                rhs=w6_sb[:, 0:512],
                start=True,
                stop=True,
            )
            nc.tensor.matmul(
                optile[:m, 512:E],
                lhsT=mean_sb[:, lo:hi],
                rhs=w6_sb[:, 512:E],
                start=True,
                stop=True,
            )
            ob = sb.tile([P, E], F32, tag=f"ob{lo}")
            nc.any.tensor_copy(ob[:m, :], optile[:m, :])
            nc.sync.dma_start(out=out.ap()[lo:hi, :], in_=ob[:m, :])

    return nc


_PROGRAM_CACHE = {}


def kernel(**inputs) -> np.ndarray:
    from concourse.bass_utils import run_bass_kernel_spmd

    img = np.asarray(inputs["img"]).astype(np.float32)
    segments = np.asarray(inputs["segments"])
    W = np.asarray(inputs["W"]).astype(np.float32)
    b = np.asarray(inputs["b"]).astype(np.float32)

    in_maps = _prep_core_inputs(img, segments, W, b)
    key = (COL_TILE, OH_BUFS, PATTERN)
    if key not in _PROGRAM_CACHE:
        _PROGRAM_CACHE[key] = _build_program()
    nc = _PROGRAM_CACHE[key]
    res = run_bass_kernel_spmd(nc, in_maps, list(range(B)))
    out = np.stack([res.results[i]["out"] for i in range(B)]).astype(np.float32)
    return out


# revision 6
# speedup vs baseline: 4.3865x; 1.0061x over previous
"""Trainium2 kernel for nn_DifferentiableSuperpixelTokenizer (segment mean of
linearly-projected pixel features).

Identity used: segment_mean(concat(img, xy) @ W + b) can be computed as
segment sums of SIX small per-pixel features (r, g, b, x, y, 1) followed by a
tiny [196, 6] @ [6, 768] projection, because the projection is linear:

    out[s] = (segsum_feat6[s] / clamp(counts[s], 1)) @ [W; b]

Sharding: data-parallel over batch (8 batches -> 8 NeuronCores), exactly as the
per-batch segment-id offsets intend; each core reduces its own batch into its
own [196, 768] slice, host stacks them.

Per core the segment reduction runs as 392 chunks of 128 pixels:
  - DVE/ACT/GPSIMD build a [128, 196] fp16 one-hot per chunk (tensor_scalar
    is_equal against a constant iota row, or Square+Relu(1-d^2) on ACT)
  - TensorE accumulates feat_chunk[128, 6].T @ onehot[128, 196] into PSUM,
    4 chunks concurrently via col-tiling (tile_position=(0, 32j))
"""
import numpy as np
from contextlib import ExitStack

# ---------------------------------------------------------------------------
# Workarounds for walrus codegen supporting a single sem-wait per instruction
# ---------------------------------------------------------------------------
import bass_rust
import concourse.mybir as mybir
import concourse.tile as tile
from concourse.tile import ScopedClock

_MAX_INST_WAITS = 1
_SELF_DROP_ENGINES = {
    mybir.EngineType.DVE: "DVE",
    mybir.EngineType.Activation: "Activation",
    mybir.EngineType.Pool: "Pool",
}


def _split_waits(ins):
    si = getattr(ins, "sync_info", None)
    if si is None:
        return []
    waits = list(si.on_wait)
    if not waits:
        return []
    self_name = _SELF_DROP_ENGINES.get(ins.engine)
    if self_name is not None:
        kept = [w for w in waits if w.ant_name.rsplit("_", 1)[0] != self_name]
    else:
        kept = waits
    head = kept[:-_MAX_INST_WAITS] if len(kept) > _MAX_INST_WAITS else []
    rest = kept[len(head):]
    if len(waits) != len(rest) or head:
        ins.sync_info = bass_rust.SyncInfo(on_wait=rest, on_update=list(si.on_update))
    return head


_orig_commit = tile.TileContext._commit_instruction


def _patched_commit(self, inst, lazy_reg_writes=True):
    head = _split_waits(inst)
    for i in range(0, len(head), _MAX_INST_WAITS):
        nop = mybir.InstNoOp(
            name=self.nc.get_next_instruction_name(),
            sync_info=mybir.SyncInfo(
                on_wait=head[i : i + _MAX_INST_WAITS], on_update=[]
            ),
            bass_nofuse=True,
            engine=inst.engine,
        )
        _orig_commit(self, nop, lazy_reg_writes=False)
    return _orig_commit(self, inst, lazy_reg_writes)


def _patched_drain_and_barrier(self, tick_clock, wait_clock):
    nc = self.nc
    drain_inst = nc.sync.drain()
    wait_clock.add_sem_waits(
        drain_inst.ins, ScopedClock({None: tick_clock.global_clock})
    )
    si = drain_inst.ins.sync_info
    waits = list(si.on_wait) if si is not None else []
    if len(waits) > 1:
        drain_inst.ins.sync_info = bass_rust.SyncInfo(on_wait=waits[:1], on_update=[])
        for w in waits[1:]:
            d2 = nc.sync.drain()
            d2.ins.sync_info = bass_rust.SyncInfo(on_wait=[w], on_update=[])
    nc.all_engine_barrier()
    assert self.sems is not None
    popped = nc._tile_sem_poison_stack.pop()
    assert popped is self._sem_poison
    nc.clear_and_free_semaphores(list(self.sems.allocated().values()))


tile.TileContext._drain_and_barrier = _patched_drain_and_barrier
tile.TileContext._commit_instruction = _patched_commit

import concourse.bass as bass  # noqa: E402

# ---------------------------------------------------------------------------
# Kernel
# ---------------------------------------------------------------------------
P, T, S, E, F = 128, 392, 196, 768, 6
H = Wimg = 224
B = 8

FP16 = mybir.dt.float16
F32 = mybir.dt.float32

PATTERN = ("dve", "dve", "dve", "dve", "dve", "act")
COL_TILE = True
OH_BUFS = 8


def _make_coords():
    x = np.arange(Wimg, dtype=np.float32) / np.float32(Wimg - 1)
    y = np.arange(H, dtype=np.float32) / np.float32(H - 1)
    xg = np.broadcast_to(x[None, :], (H, Wimg))
    yg = np.broadcast_to(y[:, None], (H, Wimg))
    return np.stack([xg.ravel(), yg.ravel()])  # [2, N] (x, y)


def _prep_core_inputs(img, segments, W, b):
    coords = _make_coords().reshape(2, P, T)
    ones = np.ones((1, P, T), np.float32)
    w6 = np.ascontiguousarray(np.concatenate([W, b[None, :]], 0).astype(np.float32))
    maps = []
    for bi in range(B):
        imgb = img[bi].reshape(3, P, T).astype(np.float32)
        feat6 = np.concatenate([imgb, coords, ones], 0)  # [6, P, T]
        feat_host = np.ascontiguousarray(
            feat6.transpose(1, 2, 0).reshape(P, T * F)
        ).astype(np.float16)
        seg_host = np.ascontiguousarray(segments[bi].reshape(P, T)).astype(np.float32)
        maps.append({"feat": feat_host, "seg": seg_host, "w6": w6})
    return maps


def _build_program(col_tile=COL_TILE, oh_bufs=OH_BUFS, pattern=PATTERN):
    nc = bass.Bass("TRN2", debug=False)
    feat = nc.dram_tensor("feat", [P, T * F], FP16, kind="ExternalInput")
    seg = nc.dram_tensor("seg", [P, T], F32, kind="ExternalInput")
    w6 = nc.dram_tensor("w6", [F, E], F32, kind="ExternalInput")
    out = nc.dram_tensor("out", [S, E], F32, kind="ExternalOutput")

    ngroups = 4 if col_tile else 1
    iota_np = np.ascontiguousarray(
        np.broadcast_to(np.arange(S, dtype=np.float16), (P, S))
    )
    iota_c = nc.inline_tensor(iota_np, name="iota_const")
    comb_np = np.zeros((P, F), np.float32)
    for j in range(ngroups):
        comb_np[32 * j : 32 * j + F, :] = np.eye(F, dtype=np.float32)
    comb_c = nc.inline_tensor(comb_np, name="comb_const")
    sel6_np = np.zeros((F, F), np.float32)
    sel6_np[F - 1, :] = 1.0
    sel6_c = nc.inline_tensor(sel6_np, name="sel6_const")

    with tile.TileContext(nc) as tc, ExitStack() as ctx:
        sb = ctx.enter_context(tc.tile_pool(name="sb", bufs=1))
        ohp = ctx.enter_context(tc.tile_pool(name="oh", bufs=oh_bufs))
        pp = ctx.enter_context(tc.tile_pool(name="psum", bufs=1, space="PSUM"))

        seg_sb = sb.tile([P, T], F32)
        nc.sync.dma_start(out=seg_sb[:], in_=seg.ap()[:, :])
        iota_sb = sb.tile([P, S], FP16)
        nc.sync.dma_start(out=iota_sb[:], in_=iota_c.ap()[:, :])
        # separate copy for ACT to avoid SBUF bank contention with DVE reads
        iota2_sb = sb.tile([P, S], FP16)
        nc.sync.dma_start(out=iota2_sb[:], in_=iota_c.ap()[:, :])
        feat_sb = sb.tile([P, T * F], FP16)
        for q in range(4):
            qs = q * (T * F // 4)
            qe = (q + 1) * (T * F // 4)
            nc.sync.dma_start(out=feat_sb[:, qs:qe], in_=feat.ap()[:, qs:qe])
        w6_sb = sb.tile([F, E], F32)
        nc.sync.dma_start(out=w6_sb[:], in_=w6.ap()[:, :])
        comb_sb = sb.tile([P, F], F32)
        nc.sync.dma_start(out=comb_sb[:], in_=comb_c.ap()[:, :])
        sel6_sb = sb.tile([F, F], F32)
        nc.sync.dma_start(out=sel6_sb[:], in_=sel6_c.ap()[:, :])

        use_act = "act" in pattern
        if use_act:
            nseg_sb = sb.tile([P, T], F32)
            nc.vector.tensor_scalar(
                out=nseg_sb[:],
                in0=seg_sb[:],
                scalar1=-1.0,
                scalar2=None,
                op0=mybir.AluOpType.mult,
            )

        acc = pp.tile([P, S], F32)
        # zero rows the col-tiled matmuls never touch (comb reads all 128
        # rows; PSUM garbage there can be NaN and 0*NaN = NaN)
        nc.vector.memset(acc[:], 0.0)
        for t in range(T):
            j = t % ngroups
            eng = pattern[t % len(pattern)]
            oh = ohp.tile([P, S], FP16, tag=f"oh_{eng}")
            if eng == "dve":
                nc.vector.tensor_scalar(
                    out=oh[:],
                    in0=iota_sb[:],
                    scalar1=seg_sb[:, t : t + 1],
                    scalar2=None,
                    op0=mybir.AluOpType.is_equal,
                )
            elif eng == "gpsimd":
                nc.gpsimd.tensor_scalar(
                    out=oh[:],
                    in0=iota_sb[:],
                    scalar1=seg_sb[:, t : t + 1],
                    scalar2=None,
                    op0=mybir.AluOpType.is_equal,
                )
            elif eng == "act":
                d2 = ohp.tile([P, S], FP16, tag="d2")
                nc.scalar.activation(
                    d2[:],
                    iota2_sb[:],
                    mybir.ActivationFunctionType.Square,
                    bias=nseg_sb[:, t : t + 1],
                )
                nc.scalar.activation(
                    oh[:],
                    d2[:],
                    mybir.ActivationFunctionType.Relu,
                    bias=1.0,
                    scale=-1.0,
                )
            else:
                raise ValueError(eng)
            nc.tensor.matmul(
                acc[32 * j : 32 * j + F, :],
                lhsT=feat_sb[:, F * t : F * (t + 1)],
                rhs=oh[:],
                start=(t < ngroups),
                stop=(t >= T - ngroups),
                tile_position=(0, 32 * j) if col_tile else None,
            )

        acc_sb = sb.tile([P, S], F32)
        nc.vector.tensor_copy(acc_sb[:], acc[:])
        sums_psum = pp.tile([F, S], F32)
        nc.tensor.matmul(
            sums_psum[:], lhsT=comb_sb[:], rhs=acc_sb[:], start=True, stop=True
        )

        cl_sb = sb.tile([F, S], F32)
        nc.vector.tensor_scalar(
            out=cl_sb[:],
            in0=sums_psum[:, :],
            scalar1=1.0,
            scalar2=None,
            op0=mybir.AluOpType.max,
        )
        recip_sb = sb.tile([F, S], F32)
        nc.vector.reciprocal(recip_sb[:], cl_sb[:])

        rb_psum = pp.tile([F, S], F32)
        nc.tensor.matmul(
            rb_psum[:], lhsT=sel6_sb[:], rhs=recip_sb[:], start=True, stop=True
        )
        sums_sb = sb.tile([F, S], F32)
        nc.vector.tensor_copy(sums_sb[:], sums_psum[:])
        mean_sb = sb.tile([F, S], F32)
        nc.vector.tensor_tensor(
            mean_sb[:], sums_sb[:], rb_psum[:], op=mybir.AluOpType.mult
        )

        for lo, hi in ((0, P), (P, S)):
            m = hi - lo
            optile = pp.tile([P, E], F32, tag=f"op{lo}")
            nc.tensor.matmul(
                optile[:m, 0:512],
                lhsT=mean_sb[:, lo:hi],
                rhs=w6_sb[:, 0:512],
                start=True,
                stop=True,
            )
            nc.tensor.matmul(
                optile[:m, 512:E],
                lhsT=mean_sb[:, lo:hi],
                rhs=w6_sb[:, 512:E],
                start=True,
                stop=True,
            )
            ob = sb.tile([P, E], F32, tag=f"ob{lo}")
            nc.any.tensor_copy(ob[:m, :], optile[:m, :])
            nc.sync.dma_start(out=out.ap()[lo:hi, :], in_=ob[:m, :])

    return nc


_PROGRAM_CACHE = {}


def kernel(**inputs) -> np.ndarray:
    from concourse.bass_utils import run_bass_kernel_spmd

    img = np.asarray(inputs["img"]).astype(np.float32)
    segments = np.asarray(inputs["segments"])
    W = np.asarray(inputs["W"]).astype(np.float32)
    b = np.asarray(inputs["b"]).astype(np.float32)

    in_maps = _prep_core_inputs(img, segments, W, b)
    key = (COL_TILE, OH_BUFS, PATTERN)
    if key not in _PROGRAM_CACHE:
        _PROGRAM_CACHE[key] = _build_program()
    nc = _PROGRAM_CACHE[key]
    res = run_bass_kernel_spmd(nc, in_maps, list(range(B)))
    out = np.stack([res.results[i]["out"] for i in range(B)]).astype(np.float32)
    return out
